# revision 25
# baseline (speedup 1.0000x reference)
"""GQA attention (B=1, L=2048, D=2048, 32 q heads, 8 kv heads, hd=64) with RoPE,
causal mask, and output projection, on 8 Trainium2 NeuronCores.

Sharding: tensor-parallel over heads. Core c owns kv head c and q heads
4c..4c+3. Each core computes its heads' attention and a partial output
projection y_c = attn_out_c @ Wo[:, 256c:256c+256].T; the host sums the 8
partials.

Device kernel layout choices (per core):
  - Q is computed transposed (Q^T: head-dim on partitions, seq on free) via
    lhsT = Wq_shard^T tiles, rhs = x^T tiles.  Wq rows are permuted on the
    host into [all 4 heads' RoPE top halves | bottom halves] so RoPE is 6
    full-width (128-partition) vector ops; per-head Q^T tiles are then
    re-assembled with SBUF->SBUF re-partitioning DMAs.
  - K^T/V^T come from one fused projection; each 128-wide tile is PE-transposed
    to natural layout, K gets RoPE in natural layout (pairs are along the free
    dim there), and K is PE-transposed back to K^T.  V stays natural with an
    appended ones column.
  - Scores are computed transposed (S^T: keys on partitions, queries on free):
    S^T = (K^T)^T-stationary @ Q^T.  Softmax needs no max subtraction (scores
    are bounded ~|6|) and no transpose of P: exp(S^T) is exactly the rhs the
    PV matmul needs.  The ones column of V yields the softmax denominator as
    row 64 of the PV accumulator for free.
  - Causal mask: off-diagonal tiles are skipped; diagonal tiles get an
    additive -1e9 triangle before exp, and the fully-masked left part is
    zeroed after exp.
  - All matmuls run as float32r (full PE rate at N>=512, ~tf32 accuracy).
"""

import numpy as np

L = 2048
D = 2048
HD = 64
N_HEADS = 32
N_KV = 8
NCORES = 8
QH = N_HEADS // N_KV  # q heads per core = 4
ROPE_THETA = 10000.0
NEG = -1e9

LG = 512  # ql group width
NG = L // LG  # 4 ql groups
NKT = L // 128  # 16 key tiles
NDT = D // 128  # 16 contraction tiles

_CACHE = {}


def _build_program(n_iter=1):
    import os
    stages = os.environ.get("K_STAGES", "full")
    import concourse.tile as tile
    import concourse.mybir as mybir
    from concourse import bacc

    f32 = mybir.dt.float32
    f32r = mybir.dt.float32r
    f16 = mybir.dt.float16
    Exp = mybir.ActivationFunctionType.Exp

    nc = bacc.Bacc("TRN2", target_bir_lowering=False, debug=False,
                   num_devices=NCORES)

    xT = nc.dram_tensor("xT", [128, NDT, L], f16, kind="ExternalInput")
    wq = nc.dram_tensor("wq", [128, NDT, 2 * 128], f16, kind="ExternalInput")
    wkv = nc.dram_tensor("wkv", [128, NDT, 128], f16, kind="ExternalInput")
    wo = nc.dram_tensor("wo", [128, 2, D], f16, kind="ExternalInput")
    cos4 = nc.dram_tensor("cos4", [128, L], f32, kind="ExternalInput")
    sin4 = nc.dram_tensor("sin4", [128, L], f32, kind="ExternalInput")
    cosT = nc.dram_tensor("cosT", [128, NKT, 32], f32, kind="ExternalInput")
    sinT = nc.dram_tensor("sinT", [128, NKT, 32], f32, kind="ExternalInput")
    # m0[:, 0:512] = -1e9 (fully-masked), m0[:, 512:640] = causal triangle
    m0 = nc.dram_tensor("m0", [128, 640], f32r, kind="ExternalInput")
    ones16 = nc.dram_tensor("ones16", [128, NKT], f32r, kind="ExternalInput")
    eye = nc.dram_tensor("eye", [128, 128], f32, kind="ExternalInput")
    eyer = nc.dram_tensor("eyer", [128, 128], f32r, kind="ExternalInput")
    y = nc.dram_tensor("y", [L, D], f16, kind="ExternalOutput")

    with tile.TileContext(nc) as tc:
        with (
            tc.tile_pool(name="consts", bufs=1) as consts,
            tc.tile_pool(name="persist", bufs=1) as persist,
            tc.tile_pool(name="r1", bufs=1) as r1p,
            tc.tile_pool(name="rb", bufs=1) as rbp,
            tc.tile_pool(name="ysb", bufs=3) as ysbp,
        ):
            # ---- constants needed for projections first ----
            wq_sb = consts.tile([128, NDT, 256], f16)
            nc.sync.dma_start(wq_sb[:], wq.ap())
            wkv_sb = consts.tile([128, NDT, 128], f16)
            nc.sync.dma_start(wkv_sb[:], wkv.ap())
            cos4_sb = consts.tile([128, L], f32)
            nc.sync.dma_start(cos4_sb[:], cos4.ap())
            sin4_sb = consts.tile([128, L], f32)
            nc.sync.dma_start(sin4_sb[:], sin4.ap())
            cosT_sb = consts.tile([128, NKT, 32], f32)
            nc.sync.dma_start(cosT_sb[:], cosT.ap())
            sinT_sb = consts.tile([128, NKT, 32], f32)
            nc.sync.dma_start(sinT_sb[:], sinT.ap())
            eye_sb = consts.tile([128, 128], f32)
            nc.sync.dma_start(eye_sb[:], eye.ap())
            eyer_sb = consts.tile([128, 128], f32r)
            nc.sync.dma_start(eyer_sb[:], eyer.ap())

            # ---- persistent intermediates ----
            qh_sb = [persist.tile([64, L], f32r, tag=f"qh{h}", name=f"qh{h}")
                     for h in range(QH)]
            kvT_sb = persist.tile([128, L], f32, tag="kvT")
            kvnat = persist.tile([128, NKT, 129], f32r, tag="kvnat")
            nc.sync.dma_start(kvnat[:, :, 128], ones16.ap())
            krot = persist.tile([128, NKT, 64], f32, tag="krot")
            kT_sb = persist.tile([64, NKT, 128], f32r, tag="kT")
            ao = [persist.tile([128, L], f16, tag=f"ao{t}", name=f"ao{t}")
                  for t in range(2)]

            for it in range(n_iter):
                # ================= phase 1: projections =================
                with (
                    tc.tile_pool(name="xin", bufs=3) as xin,
                    tc.tile_pool(name="ropetmp", bufs=4) as ropetmp,
                    tc.tile_pool(name="qrot", bufs=4) as qrotp,
                    tc.tile_pool(name="krtmp", bufs=2) as krtmp,
                    tc.tile_pool(name="proj_ps", bufs=6, space="PSUM") as proj_ps,
                    tc.tile_pool(name="tp_ps", bufs=2, space="PSUM") as tp_ps,
                ):
                    for g in range(NG):
                        gsl = slice(g * LG, (g + 1) * LG)
                        ps_qa = proj_ps.tile([128, LG], f32, tag="ps_q")
                        ps_qb = proj_ps.tile([128, LG], f32, tag="ps_q")
                        ps_kv = proj_ps.tile([128, LG], f32, tag="ps_q")
                        for ob in range(4):
                            xt = xin.tile([128, 4, LG], f16, tag="xt")
                            nc.sync.dma_start(xt[:], xT.ap()[:, 4 * ob:4 * ob + 4, gsl])
                            for oi in range(4):
                                o = 4 * ob + oi
                                st, sp = (o == 0), (o == NDT - 1)
                                nc.tensor.matmul(ps_qa[:], wq_sb[:, o, 0:128],
                                                 xt[:, oi, :], start=st, stop=sp)
                                nc.tensor.matmul(ps_qb[:], wq_sb[:, o, 128:256],
                                                 xt[:, oi, :], start=st, stop=sp)
                                nc.tensor.matmul(ps_kv[:], wkv_sb[:, o, :],
                                                 xt[:, oi, :], start=st, stop=sp)

                        # RoPE on Q (A = tops, B = bottoms)
                        t_a = ropetmp.tile([128, LG], f32, tag="t_a")
                        nc.vector.tensor_mul(out=t_a[:], in0=ps_qa[:], in1=cos4_sb[:, gsl])
                        t_b = ropetmp.tile([128, LG], f32, tag="t_b")
                        nc.vector.tensor_mul(out=t_b[:], in0=ps_qb[:], in1=sin4_sb[:, gsl])
                        qa_r = qrotp.tile([128, LG], f32r, tag="qa_r")
                        nc.vector.tensor_sub(out=qa_r[:], in0=t_a[:], in1=t_b[:])
                        t_c = ropetmp.tile([128, LG], f32, tag="t_a")
                        nc.vector.tensor_mul(out=t_c[:], in0=ps_qa[:], in1=sin4_sb[:, gsl])
                        t_d = ropetmp.tile([128, LG], f32, tag="t_b")
                        nc.vector.tensor_mul(out=t_d[:], in0=ps_qb[:], in1=cos4_sb[:, gsl])
                        qb_r = qrotp.tile([128, LG], f32r, tag="qb_r")
                        nc.vector.tensor_add(out=qb_r[:], in0=t_c[:], in1=t_d[:])
                        for j in range(QH):
                            nc.gpsimd.tensor_copy(out=qh_sb[j][0:32, gsl],
                                                  in_=qa_r[32 * j:32 * j + 32, :])
                            nc.gpsimd.tensor_copy(out=qh_sb[j][32:64, gsl],
                                                  in_=qb_r[32 * j:32 * j + 32, :])

                        # K/V -> natural layout; K RoPE; K back to K^T
                        nc.vector.tensor_copy(out=kvT_sb[:, gsl], in_=ps_kv[:])
                        for ki in range(4 * g, 4 * g + 4):
                            tp = tp_ps.tile([128, 128], f32, tag="tp")
                            nc.tensor.transpose(tp[:], kvT_sb[:, ki * 128:(ki + 1) * 128],
                                                eye_sb[:])
                            nc.vector.tensor_copy(out=kvnat[:, ki, 0:128], in_=tp[:])
                        ksl = slice(4 * g, 4 * g + 4)
                        u1 = krtmp.tile([128, 4, 32], f32, tag="u1")
                        nc.vector.tensor_mul(out=u1[:], in0=kvnat[:, ksl, 0:32],
                                             in1=cosT_sb[:, ksl, :])
                        u2 = krtmp.tile([128, 4, 32], f32, tag="u2")
                        nc.vector.tensor_mul(out=u2[:], in0=kvnat[:, ksl, 32:64],
                                             in1=sinT_sb[:, ksl, :])
                        nc.vector.tensor_sub(out=krot[:, ksl, 0:32], in0=u1[:], in1=u2[:])
                        u3 = krtmp.tile([128, 4, 32], f32, tag="u1")
                        nc.vector.tensor_mul(out=u3[:], in0=kvnat[:, ksl, 0:32],
                                             in1=sinT_sb[:, ksl, :])
                        u4 = krtmp.tile([128, 4, 32], f32, tag="u2")
                        nc.vector.tensor_mul(out=u4[:], in0=kvnat[:, ksl, 32:64],
                                             in1=cosT_sb[:, ksl, :])
                        nc.vector.tensor_add(out=krot[:, ksl, 32:64], in0=u3[:], in1=u4[:])
                        for ki in range(4 * g, 4 * g + 4):
                            tb = tp_ps.tile([128, 128], f32, tag="tp")
                            nc.tensor.transpose(tb[0:64, :], krot[:, ki, :], eye_sb[:])
                            nc.vector.tensor_copy(out=kT_sb[:, ki, :], in_=tb[0:64, :])

                # ---- remaining constants (first used below) ----
                if it == 0:
                    wo_sb = consts.tile([128, 2, D], f16)
                    nc.sync.dma_start(wo_sb[:], wo.ap())
                    m0_sb = consts.tile([128, 640], f32r)
                    nc.sync.dma_start(m0_sb[:], m0.ap())

                if stages == "proj":
                    continue
                # ================= phase 2: attention + Wo ==============
                LW = 2 * LG  # 1024-wide attention groups
                with (
                    tc.tile_pool(name="pt", bufs=3) as ptp,
                    tc.tile_pool(name="st_ps", bufs=2, space="PSUM") as st_ps,
                    tc.tile_pool(name="pv_ps", bufs=2, space="PSUM") as pv_ps,
                ):
                    for j in range(L // LW):
                        jsl = slice(j * LW, (j + 1) * LW)
                        nkt = 8 * j + 8
                        for h in range(QH):
                            pv = pv_ps.tile([65, LW], f32, tag="pv")
                            for ki in range(nkt):
                                # live columns of this 1024 group
                                off = max(0, 128 * ki - j * LW)
                                stp = st_ps.tile([128, LW], f32, tag="st")
                                for h2 in range(2):
                                    lo = max(off, h2 * LG)
                                    if lo >= (h2 + 1) * LG:
                                        continue
                                    s2 = slice(lo, (h2 + 1) * LG)
                                    nc.tensor.matmul(
                                        stp[:, s2], kT_sb[:, ki, :],
                                        qh_sb[h][:, j * LW + lo:j * LW + (h2 + 1) * LG],
                                        start=True, stop=True)
                                if off < LW and 128 * ki >= j * LW:
                                    dsl = slice(off, off + 128)
                                    nc.tensor.matmul(stp[:, dsl], eyer_sb[:],
                                                     m0_sb[:, 512:640],
                                                     start=False, stop=True,
                                                     skip_group_check=True)
                                pt = ptp.tile([128, LW], f32r, tag="pt")
                                nc.scalar.activation(pt[:, off:LW], stp[:, off:LW], Exp)
                                for h2 in range(2):
                                    lo = max(off, h2 * LG)
                                    if lo >= (h2 + 1) * LG:
                                        continue
                                    s2 = slice(lo, (h2 + 1) * LG)
                                    nc.tensor.matmul(
                                        pv[:, s2], kvnat[:, ki, 64:129], pt[:, s2],
                                        start=(ki == 0),
                                        stop=(ki == 8 * j + 4 * h2 + 3))
                            # normalize via ones-row denominator
                            r1 = r1p.tile([1, LW], f32, tag="r1")
                            nc.vector.reciprocal(r1[:], pv[64:65, :])
                            rb = rbp.tile([64, LW], f32, tag="rb")
                            nc.gpsimd.partition_broadcast(rb[:], r1[:])
                            nc.vector.tensor_mul(
                                out=ao[h // 2][64 * (h % 2):64 * (h % 2) + 64, jsl],
                                in0=pv[0:64, :], in1=rb[:])

                        # output projection for l rows of this 1024 group
                        if stages == "nowo":
                            continue
                        for m in range(8 * j, 8 * j + 8):
                            msl = slice(m * 128, (m + 1) * 128)
                            ys = ysbp.tile([128, D], f16, tag="ys")
                            for gn in range(NG):
                                nsl = slice(gn * LG, (gn + 1) * LG)
                                yp = st_ps.tile([128, LG], f32, tag="st")
                                nc.tensor.matmul(yp[:], ao[0][:, msl], wo_sb[:, 0, nsl],
                                                 start=True, stop=False)
                                nc.tensor.matmul(yp[:], ao[1][:, msl], wo_sb[:, 1, nsl],
                                                 start=False, stop=True)
                                if gn % 2 == 0:
                                    nc.scalar.copy(ys[:, nsl], yp[:])
                                else:
                                    nc.vector.tensor_copy(out=ys[:, nsl], in_=yp[:])
                            nc.sync.dma_start(y.ap()[msl, :], ys[:])

    nc.compile()
    return nc


def _host_prep(x, attn_scale, Wq, Wk, Wv, Wo):
    """Build the 8 per-core input maps."""
    xT = np.ascontiguousarray(x.reshape(L, D).T)  # [D, L]
    xT_dev = np.ascontiguousarray(xT.reshape(NDT, 128, L).transpose(1, 0, 2))

    pos = np.arange(L, dtype=np.float64)
    inv_freq = 1.0 / (ROPE_THETA ** (np.arange(0, HD, 2, dtype=np.float64) / HD))
    ang = pos[:, None] * inv_freq[None, :]  # [L, 32]
    cos = np.cos(ang).astype(np.float32)  # [L, 32]
    sin = np.sin(ang).astype(np.float32)
    cos4 = np.ascontiguousarray(np.tile(cos.T, (4, 1)))  # [128, L]
    sin4 = np.ascontiguousarray(np.tile(sin.T, (4, 1)))
    cosT = np.ascontiguousarray(cos.reshape(NKT, 128, 32).transpose(1, 0, 2))
    sinT = np.ascontiguousarray(sin.reshape(NKT, 128, 32).transpose(1, 0, 2))

    p = np.arange(128)
    tri = np.where(p[:, None] <= p[None, :], 0.0, NEG).astype(np.float32)
    m0 = np.concatenate([np.full((128, 512), NEG, np.float32), tri], axis=1)
    eye = np.eye(128, dtype=np.float32)

    kscale = float(attn_scale.reshape(-1)[0]) * HD ** -0.5

    in_maps = []
    for c in range(NCORES):
        rows_a = [Wq[256 * c + 64 * j:256 * c + 64 * j + 32] for j in range(QH)]
        rows_b = [Wq[256 * c + 64 * j + 32:256 * c + 64 * j + 64] for j in range(QH)]
        WqAB = np.concatenate(rows_a + rows_b, axis=0)  # [256, D]
        wq_dev = np.ascontiguousarray(
            WqAB.T.reshape(NDT, 128, 256).transpose(1, 0, 2))

        Wk_c = Wk[64 * c:64 * c + 64] * kscale
        Wv_c = Wv[64 * c:64 * c + 64]
        WKV = np.concatenate([Wk_c, Wv_c], axis=0)  # [128, D]
        wkv_dev = np.ascontiguousarray(
            WKV.T.reshape(NDT, 128, 128).transpose(1, 0, 2))

        WoT_c = Wo[:, 256 * c:256 * c + 256].T  # [256, D]
        wo_dev = np.ascontiguousarray(
            WoT_c.reshape(2, 128, D).transpose(1, 0, 2))

        in_maps.append({
            "xT": xT_dev.astype(np.float16), "wq": wq_dev.astype(np.float16),
            "wkv": wkv_dev.astype(np.float16), "wo": wo_dev.astype(np.float16),
            "cos4": cos4, "sin4": sin4, "cosT": cosT, "sinT": sinT,
            "m0": m0, "eye": eye, "eyer": eye,
            "ones16": np.ones((128, NKT), np.float32),
        })
    return in_maps


def _get_program(n_iter=1):
    import os
    key = f"nc{n_iter}-{os.environ.get('K_STAGES', 'full')}"
    if key not in _CACHE:
        _CACHE[key] = _build_program(n_iter)
    return _CACHE[key]


def run(inputs, trace=False):
    """Run on 8 NeuronCores; returns (y_full, BassKernelResults)."""
    from concourse import bass_utils

    in_maps = _host_prep(inputs["x"], inputs["attn_scale"], inputs["Wq"],
                         inputs["Wk"], inputs["Wv"], inputs["Wo"])
    nc = _get_program()
    res = bass_utils.run_bass_kernel_spmd(
        nc, in_maps, core_ids=list(range(NCORES)), trace=trace)
    parts = np.stack([res.results[c]["y"] for c in range(NCORES)])
    y = parts.sum(axis=0, dtype=np.float64).astype(np.float32)
    return y.reshape(1, L, D), res


def kernel(**inputs):
    y, _ = run(inputs, trace=False)
    return y


# revision 26
# speedup vs baseline: 1.1801x; 1.1801x over previous
"""GQA attention (B=1, L=2048, D=2048, 32 q heads, 8 kv heads, hd=64) with RoPE,
causal mask, and output projection, on 8 Trainium2 NeuronCores.

Sharding: tensor-parallel over heads. Core c owns kv head c and q heads
4c..4c+3. Each core computes its heads' attention and a partial output
projection y_c = attn_out_c @ Wo[:, 256c:256c+256].T; the host sums the 8
partials.

Device kernel layout choices (per core):
  - Q is computed transposed (Q^T: head-dim on partitions, seq on free) via
    lhsT = Wq_shard^T tiles, rhs = x^T tiles.  Wq rows are permuted on the
    host into [all 4 heads' RoPE top halves | bottom halves] so RoPE is 6
    full-width (128-partition) vector ops; per-head Q^T tiles are then
    re-assembled with SBUF->SBUF re-partitioning DMAs.
  - K^T/V^T come from one fused projection; each 128-wide tile is PE-transposed
    to natural layout, K gets RoPE in natural layout (pairs are along the free
    dim there), and K is PE-transposed back to K^T.  V stays natural with an
    appended ones column.
  - Scores are computed transposed (S^T: keys on partitions, queries on free):
    S^T = (K^T)^T-stationary @ Q^T.  Softmax needs no max subtraction (scores
    are bounded ~|6|) and no transpose of P: exp(S^T) is exactly the rhs the
    PV matmul needs.  The ones column of V yields the softmax denominator as
    row 64 of the PV accumulator for free.
  - Causal mask: off-diagonal tiles are skipped; diagonal tiles get an
    additive -1e9 triangle before exp, and the fully-masked left part is
    zeroed after exp.
  - All matmuls run as float32r (full PE rate at N>=512, ~tf32 accuracy).
"""

import numpy as np

L = 2048
D = 2048
HD = 64
N_HEADS = 32
N_KV = 8
NCORES = 8
QH = N_HEADS // N_KV  # q heads per core = 4
ROPE_THETA = 10000.0
NEG = -1e9

LG = 512  # ql group width
NG = L // LG  # 4 ql groups
NKT = L // 128  # 16 key tiles
NDT = D // 128  # 16 contraction tiles

_CACHE = {}


def _build_program(n_iter=1):
    import os
    stages = os.environ.get("K_STAGES", "full")
    import concourse.tile as tile
    import concourse.mybir as mybir
    from concourse import bacc

    f32 = mybir.dt.float32
    f32r = mybir.dt.float32r
    f16 = mybir.dt.float16
    Exp = mybir.ActivationFunctionType.Exp

    nc = bacc.Bacc("TRN2", target_bir_lowering=False, debug=False,
                   num_devices=NCORES)

    xT = nc.dram_tensor("xT", [128, NDT, L], f16, kind="ExternalInput")
    wq = nc.dram_tensor("wq", [128, NDT, 2 * 128], f16, kind="ExternalInput")
    wkv = nc.dram_tensor("wkv", [128, NDT, 128], f16, kind="ExternalInput")
    wo = nc.dram_tensor("wo", [128, 2, D], f16, kind="ExternalInput")
    cos4 = nc.dram_tensor("cos4", [128, L], f32, kind="ExternalInput")
    sin4 = nc.dram_tensor("sin4", [128, L], f32, kind="ExternalInput")
    cosT = nc.dram_tensor("cosT", [128, NKT, 32], f32, kind="ExternalInput")
    sinT = nc.dram_tensor("sinT", [128, NKT, 32], f32, kind="ExternalInput")
    # m0[:, 0:512] = -1e9 (fully-masked), m0[:, 512:640] = causal triangle
    m0 = nc.dram_tensor("m0", [128, 640], f32r, kind="ExternalInput")
    ones16 = nc.dram_tensor("ones16", [128, NKT], f32r, kind="ExternalInput")
    eye = nc.dram_tensor("eye", [128, 128], f32, kind="ExternalInput")
    eyer = nc.dram_tensor("eyer", [128, 128], f32r, kind="ExternalInput")
    y = nc.dram_tensor("y", [L, D], f16, kind="ExternalOutput")

    with tile.TileContext(nc) as tc:
        with (
            tc.tile_pool(name="consts", bufs=1) as consts,
            tc.tile_pool(name="persist", bufs=1) as persist,
            tc.tile_pool(name="r1", bufs=1) as r1p,
            tc.tile_pool(name="rb", bufs=1) as rbp,
            tc.tile_pool(name="ysb", bufs=3) as ysbp,
        ):
            # ---- constants needed for projections first ----
            wq_sb = consts.tile([128, NDT, 256], f16)
            nc.sync.dma_start(wq_sb[:], wq.ap())
            wkv_sb = consts.tile([128, NDT, 128], f16)
            nc.sync.dma_start(wkv_sb[:], wkv.ap())
            cos4_sb = consts.tile([128, L], f32)
            nc.sync.dma_start(cos4_sb[:], cos4.ap())
            sin4_sb = consts.tile([128, L], f32)
            nc.sync.dma_start(sin4_sb[:], sin4.ap())
            cosT_sb = consts.tile([128, NKT, 32], f32)
            nc.sync.dma_start(cosT_sb[:], cosT.ap())
            sinT_sb = consts.tile([128, NKT, 32], f32)
            nc.sync.dma_start(sinT_sb[:], sinT.ap())
            eye_sb = consts.tile([128, 128], f32)
            nc.sync.dma_start(eye_sb[:], eye.ap())
            eyer_sb = consts.tile([128, 128], f32r)
            nc.sync.dma_start(eyer_sb[:], eyer.ap())

            # ---- persistent intermediates ----
            qh_sb = [persist.tile([64, L], f32r, tag=f"qh{h}", name=f"qh{h}")
                     for h in range(QH)]
            kvT_sb = persist.tile([128, L], f32, tag="kvT")
            kvnat = persist.tile([128, NKT, 129], f32r, tag="kvnat")
            nc.sync.dma_start(kvnat[:, :, 128], ones16.ap())
            krot = persist.tile([128, NKT, 64], f32, tag="krot")
            kT_sb = persist.tile([64, NKT, 128], f32r, tag="kT")
            ao = [persist.tile([128, L], f16, tag=f"ao{t}", name=f"ao{t}")
                  for t in range(2)]

            for it in range(n_iter):
                # ================= phase 1: projections =================
                with (
                    tc.tile_pool(name="xin", bufs=3) as xin,
                    tc.tile_pool(name="ropetmp", bufs=4) as ropetmp,
                    tc.tile_pool(name="qrot", bufs=4) as qrotp,
                    tc.tile_pool(name="krtmp", bufs=2) as krtmp,
                    tc.tile_pool(name="proj_ps", bufs=6, space="PSUM") as proj_ps,
                    tc.tile_pool(name="tp_ps", bufs=2, space="PSUM") as tp_ps,
                ):
                    for g in range(NG):
                        gsl = slice(g * LG, (g + 1) * LG)
                        ps_qa = proj_ps.tile([128, LG], f32, tag="ps_q")
                        ps_qb = proj_ps.tile([128, LG], f32, tag="ps_q")
                        ps_kv = proj_ps.tile([128, LG], f32, tag="ps_q")
                        for ob in range(4):
                            xt = xin.tile([128, 4, LG], f16, tag="xt")
                            nc.sync.dma_start(xt[:], xT.ap()[:, 4 * ob:4 * ob + 4, gsl])
                            for oi in range(4):
                                o = 4 * ob + oi
                                st, sp = (o == 0), (o == NDT - 1)
                                nc.tensor.matmul(ps_qa[:], wq_sb[:, o, 0:128],
                                                 xt[:, oi, :], start=st, stop=sp)
                                nc.tensor.matmul(ps_qb[:], wq_sb[:, o, 128:256],
                                                 xt[:, oi, :], start=st, stop=sp)
                                nc.tensor.matmul(ps_kv[:], wkv_sb[:, o, :],
                                                 xt[:, oi, :], start=st, stop=sp)

                        # RoPE on Q (A = tops, B = bottoms)
                        t_a = ropetmp.tile([128, LG], f32, tag="t_a")
                        nc.vector.tensor_mul(out=t_a[:], in0=ps_qa[:], in1=cos4_sb[:, gsl])
                        t_b = ropetmp.tile([128, LG], f32, tag="t_b")
                        nc.vector.tensor_mul(out=t_b[:], in0=ps_qb[:], in1=sin4_sb[:, gsl])
                        qa_r = qrotp.tile([128, LG], f32r, tag="qa_r")
                        nc.vector.tensor_sub(out=qa_r[:], in0=t_a[:], in1=t_b[:])
                        t_c = ropetmp.tile([128, LG], f32, tag="t_a")
                        nc.vector.tensor_mul(out=t_c[:], in0=ps_qa[:], in1=sin4_sb[:, gsl])
                        t_d = ropetmp.tile([128, LG], f32, tag="t_b")
                        nc.vector.tensor_mul(out=t_d[:], in0=ps_qb[:], in1=cos4_sb[:, gsl])
                        qb_r = qrotp.tile([128, LG], f32r, tag="qb_r")
                        nc.vector.tensor_add(out=qb_r[:], in0=t_c[:], in1=t_d[:])
                        for j in range(QH):
                            nc.gpsimd.tensor_copy(out=qh_sb[j][0:32, gsl],
                                                  in_=qa_r[32 * j:32 * j + 32, :])
                            nc.gpsimd.tensor_copy(out=qh_sb[j][32:64, gsl],
                                                  in_=qb_r[32 * j:32 * j + 32, :])

                        # K/V -> natural layout; K RoPE; K back to K^T
                        nc.vector.tensor_copy(out=kvT_sb[:, gsl], in_=ps_kv[:])
                        for ki in range(4 * g, 4 * g + 4):
                            tp = tp_ps.tile([128, 128], f32, tag="tp")
                            nc.tensor.transpose(tp[:], kvT_sb[:, ki * 128:(ki + 1) * 128],
                                                eye_sb[:])
                            nc.vector.tensor_copy(out=kvnat[:, ki, 0:128], in_=tp[:])
                        ksl = slice(4 * g, 4 * g + 4)
                        u1 = krtmp.tile([128, 4, 32], f32, tag="u1")
                        nc.vector.tensor_mul(out=u1[:], in0=kvnat[:, ksl, 0:32],
                                             in1=cosT_sb[:, ksl, :])
                        u2 = krtmp.tile([128, 4, 32], f32, tag="u2")
                        nc.vector.tensor_mul(out=u2[:], in0=kvnat[:, ksl, 32:64],
                                             in1=sinT_sb[:, ksl, :])
                        nc.vector.tensor_sub(out=krot[:, ksl, 0:32], in0=u1[:], in1=u2[:])
                        u3 = krtmp.tile([128, 4, 32], f32, tag="u1")
                        nc.vector.tensor_mul(out=u3[:], in0=kvnat[:, ksl, 0:32],
                                             in1=sinT_sb[:, ksl, :])
                        u4 = krtmp.tile([128, 4, 32], f32, tag="u2")
                        nc.vector.tensor_mul(out=u4[:], in0=kvnat[:, ksl, 32:64],
                                             in1=cosT_sb[:, ksl, :])
                        nc.vector.tensor_add(out=krot[:, ksl, 32:64], in0=u3[:], in1=u4[:])
                        for ki in range(4 * g, 4 * g + 4):
                            tb = tp_ps.tile([128, 128], f32, tag="tp")
                            nc.tensor.transpose(tb[0:64, :], krot[:, ki, :], eye_sb[:])
                            nc.vector.tensor_copy(out=kT_sb[:, ki, :], in_=tb[0:64, :])

                # ---- remaining constants (first used below) ----
                if it == 0:
                    wo_sb = consts.tile([128, 2, D], f16)
                    nc.sync.dma_start(wo_sb[:], wo.ap())
                    m0_sb = consts.tile([128, 640], f32r)
                    nc.sync.dma_start(m0_sb[:], m0.ap())

                if stages == "proj":
                    continue
                # ================= phase 2: attention + Wo ==============
                LW = 2 * LG  # 1024-wide attention groups
                with (
                    tc.tile_pool(name="pt", bufs=3) as ptp,
                    tc.tile_pool(name="st_ps", bufs=2, space="PSUM") as st_ps,
                    tc.tile_pool(name="pv_ps", bufs=2, space="PSUM") as pv_ps,
                ):
                    for j in range(L // LW):
                        jsl = slice(j * LW, (j + 1) * LW)
                        nkt = 8 * j + 8
                        for h in range(QH):
                            pv = pv_ps.tile([65, LW], f32, tag="pv")
                            for ki in range(nkt):
                                # live columns of this 1024 group
                                off = max(0, 128 * ki - j * LW)
                                stp = st_ps.tile([128, LW], f32, tag="st")
                                for h2 in range(2):
                                    lo = max(off, h2 * LG)
                                    if lo >= (h2 + 1) * LG:
                                        continue
                                    s2 = slice(lo, (h2 + 1) * LG)
                                    nc.tensor.matmul(
                                        stp[:, s2], kT_sb[:, ki, :],
                                        qh_sb[h][:, j * LW + lo:j * LW + (h2 + 1) * LG],
                                        start=True, stop=True)
                                if off < LW and 128 * ki >= j * LW:
                                    dsl = slice(off, off + 128)
                                    nc.vector.tensor_add(out=stp[:, dsl],
                                                         in0=stp[:, dsl],
                                                         in1=m0_sb[:, 512:640].bitcast(f32))
                                pt = ptp.tile([128, LW], f32r, tag="pt")
                                nc.scalar.activation(pt[:, off:LW], stp[:, off:LW], Exp)
                                for h2 in range(2):
                                    lo = max(off, h2 * LG)
                                    if lo >= (h2 + 1) * LG:
                                        continue
                                    s2 = slice(lo, (h2 + 1) * LG)
                                    nc.tensor.matmul(
                                        pv[:, s2], kvnat[:, ki, 64:129], pt[:, s2],
                                        start=(ki == 0),
                                        stop=(ki == 8 * j + 4 * h2 + 3))
                            # normalize via ones-row denominator
                            r1 = r1p.tile([1, LW], f32, tag="r1")
                            nc.vector.reciprocal(r1[:], pv[64:65, :])
                            rb = rbp.tile([64, LW], f32, tag="rb")
                            nc.gpsimd.partition_broadcast(rb[:], r1[:])
                            nc.vector.tensor_mul(
                                out=ao[h // 2][64 * (h % 2):64 * (h % 2) + 64, jsl],
                                in0=pv[0:64, :], in1=rb[:])

                        # output projection for l rows of this 1024 group
                        if stages == "nowo":
                            continue
                        for m in range(8 * j, 8 * j + 8):
                            msl = slice(m * 128, (m + 1) * 128)
                            ys = ysbp.tile([128, D], f16, tag="ys")
                            for gn in range(NG):
                                nsl = slice(gn * LG, (gn + 1) * LG)
                                yp = st_ps.tile([128, LG], f32, tag="st")
                                nc.tensor.matmul(yp[:], ao[0][:, msl], wo_sb[:, 0, nsl],
                                                 start=True, stop=False)
                                nc.tensor.matmul(yp[:], ao[1][:, msl], wo_sb[:, 1, nsl],
                                                 start=False, stop=True)
                                if gn % 2 == 0:
                                    nc.scalar.copy(ys[:, nsl], yp[:])
                                else:
                                    nc.vector.tensor_copy(out=ys[:, nsl], in_=yp[:])
                            nc.sync.dma_start(y.ap()[msl, :], ys[:])

    nc.compile()
    return nc


def _host_prep(x, attn_scale, Wq, Wk, Wv, Wo):
    """Build the 8 per-core input maps."""
    xT = np.ascontiguousarray(x.reshape(L, D).T)  # [D, L]
    xT_dev = np.ascontiguousarray(xT.reshape(NDT, 128, L).transpose(1, 0, 2))

    pos = np.arange(L, dtype=np.float64)
    inv_freq = 1.0 / (ROPE_THETA ** (np.arange(0, HD, 2, dtype=np.float64) / HD))
    ang = pos[:, None] * inv_freq[None, :]  # [L, 32]
    cos = np.cos(ang).astype(np.float32)  # [L, 32]
    sin = np.sin(ang).astype(np.float32)
    cos4 = np.ascontiguousarray(np.tile(cos.T, (4, 1)))  # [128, L]
    sin4 = np.ascontiguousarray(np.tile(sin.T, (4, 1)))
    cosT = np.ascontiguousarray(cos.reshape(NKT, 128, 32).transpose(1, 0, 2))
    sinT = np.ascontiguousarray(sin.reshape(NKT, 128, 32).transpose(1, 0, 2))

    p = np.arange(128)
    tri = np.where(p[:, None] <= p[None, :], 0.0, NEG).astype(np.float32)
    m0 = np.concatenate([np.full((128, 512), NEG, np.float32), tri], axis=1)
    eye = np.eye(128, dtype=np.float32)

    kscale = float(attn_scale.reshape(-1)[0]) * HD ** -0.5

    in_maps = []
    for c in range(NCORES):
        rows_a = [Wq[256 * c + 64 * j:256 * c + 64 * j + 32] for j in range(QH)]
        rows_b = [Wq[256 * c + 64 * j + 32:256 * c + 64 * j + 64] for j in range(QH)]
        WqAB = np.concatenate(rows_a + rows_b, axis=0)  # [256, D]
        wq_dev = np.ascontiguousarray(
            WqAB.T.reshape(NDT, 128, 256).transpose(1, 0, 2))

        Wk_c = Wk[64 * c:64 * c + 64] * kscale
        Wv_c = Wv[64 * c:64 * c + 64]
        WKV = np.concatenate([Wk_c, Wv_c], axis=0)  # [128, D]
        wkv_dev = np.ascontiguousarray(
            WKV.T.reshape(NDT, 128, 128).transpose(1, 0, 2))

        WoT_c = Wo[:, 256 * c:256 * c + 256].T  # [256, D]
        wo_dev = np.ascontiguousarray(
            WoT_c.reshape(2, 128, D).transpose(1, 0, 2))

        in_maps.append({
            "xT": xT_dev.astype(np.float16), "wq": wq_dev.astype(np.float16),
            "wkv": wkv_dev.astype(np.float16), "wo": wo_dev.astype(np.float16),
            "cos4": cos4, "sin4": sin4, "cosT": cosT, "sinT": sinT,
            "m0": m0, "eye": eye, "eyer": eye,
            "ones16": np.ones((128, NKT), np.float32),
        })
    return in_maps


def _get_program(n_iter=1):
    import os
    key = f"nc{n_iter}-{os.environ.get('K_STAGES', 'full')}"
    if key not in _CACHE:
        _CACHE[key] = _build_program(n_iter)
    return _CACHE[key]


def run(inputs, trace=False):
    """Run on 8 NeuronCores; returns (y_full, BassKernelResults)."""
    from concourse import bass_utils

    in_maps = _host_prep(inputs["x"], inputs["attn_scale"], inputs["Wq"],
                         inputs["Wk"], inputs["Wv"], inputs["Wo"])
    nc = _get_program()
    res = bass_utils.run_bass_kernel_spmd(
        nc, in_maps, core_ids=list(range(NCORES)), trace=trace)
    parts = np.stack([res.results[c]["y"] for c in range(NCORES)])
    y = parts.sum(axis=0, dtype=np.float64).astype(np.float32)
    return y.reshape(1, L, D), res


def kernel(**inputs):
    y, _ = run(inputs, trace=False)
    return y
